# revision 1
# baseline (speedup 1.0000x reference)
"""DGCNN-style GNN (2x dynamic-kNN EdgeConv + global pool + MLP head) on 8 Trainium2
NeuronCores, data-parallel over the 512 graphs (64 graphs per core).

Self-contained: hardcodes all shapes; host side only does layout prep (transpose /
tiling / block-diag packing of weights) and sharding.
"""

import sys

for _p in ("/opt/trn_rl_repo",):
    if _p not in sys.path:
        sys.path.append(_p)

from contextlib import ExitStack

import numpy as np

import concourse.bass as bass
import concourse.tile as tile
from concourse import bacc, mybir
from concourse.bass_utils import run_bass_kernel_spmd

F32 = mybir.dt.float32
U16 = mybir.dt.uint16
I16 = mybir.dt.int16
AF = mybir.ActivationFunctionType
ALU = mybir.AluOpType
AX = mybir.AxisListType

B, N, K = 512, 256, 20
NCORES = 8
GPC = B // NCORES  # graphs per core = 64
NEG = -1.0e30


# ---------------------------------------------------------------------------
# Bass program builder (identical program for every core; all data via inputs)
# ---------------------------------------------------------------------------

def build_program(sets: int = 8):
    """Build the per-core Bass program. `sets` = number of 8-graph sets (8 => 64
    graphs). Returns the compiled Bacc object."""
    G = 8 * sets
    nc = bacc.Bacc("TRN2", target_bir_lowering=False, debug=False)

    def din(name, shape, dtype=F32):
        return nc.declare_dram_parameter(name, list(shape), dtype, isOutput=False)

    # -------------------- DRAM parameters --------------------
    xtf_d = din("xtf", [128, 256])          # [2g+f, j] = x[256g+j, f] (g<64)
    a4_d = din("a4all", [4, 16384])         # rows: x0, x1, ones, zeros
    b4_d = din("b4all", [4, 16384])         # rows: x0, x1, zeros(->-sq/2), ones
    cwrep_d = din("cwrep", [2, 128])        # tile8(c1_w0[:2]-c1_w0[2:4])
    bwrep_d = din("bwrep", [2, 128])        # tile8(c1_w0[2:4])
    b0rep_d = din("b0rep", [128, 1])        # tile8(c1_b0)
    w1bd_d = din("w1bd", [128, 128])        # blkdiag8(c1_w1)
    b1rep_d = din("b1rep", [128, 1])
    w2bd_d = din("w2bd", [128, 128])        # blkdiag8(c1_w2)
    b2rep_d = din("b2rep", [128, 1])
    bdgf_d = din("bdgf", [128, 64])         # [2g+f, g] = -0.5
    nh16_d = din("neghalf16", [16, 1])      # -0.5
    ones_d = din("onesrow", [1, 2048])
    zeros_d = din("zerorow", [1, 2048])
    iota_d = din("iotaidx", [128, 40], I16)  # [:, 20t:20t+20] = 128t+p (idx of self)
    qarep_d = din("wb2repA", [16, 128])     # tile8(c2_w0[16:32, :16])
    qbrep_d = din("wb2repB", [16, 128])     # tile8(c2_w0[16:32, 16:])
    wd2aug_d = din("wd2aug", [18, 32])      # [c2_w0[:16]-c2_w0[16:32]; 0; c2_b0]
    w1l_d = din("w1l", [48, 128])           # lin1_w
    b1l_d = din("b1l", [128, 1])            # lin1_b
    mw0_d = din("mw0", [128, 64])
    mb0_d = din("mb0", [64, 1])
    mw1_d = din("mw1", [64, 64])
    mb1_d = din("mb1", [64, 1])
    mw2_d = din("mw2", [64, 1])
    mb2_d = din("mb2", [1, 1])
    out_d = nc.declare_dram_parameter("out", [1, G], F32, isOutput=True)

    with tile.TileContext(nc) as tc, ExitStack() as ctx:
        P = lambda **kw: ctx.enter_context(tc.tile_pool(**kw))
        wp = P(name="weights", bufs=1)

        def load(dram, shape, dtype=F32):
            t = wp.tile(list(shape), dtype, tag=dram.name)
            nc.sync.dma_start(t[:], dram.ap())
            return t

        xtf = load(xtf_d, [128, 256])
        cwrep = load(cwrep_d, [2, 128])
        bwrep = load(bwrep_d, [2, 128])
        b0rep = load(b0rep_d, [128, 1])
        w1bd = load(w1bd_d, [128, 128])
        b1rep = load(b1rep_d, [128, 1])
        w2bd = load(w2bd_d, [128, 128])
        b2rep = load(b2rep_d, [128, 1])
        bdgf = load(bdgf_d, [128, 64])
        nh16 = load(nh16_d, [16, 1])
        onesr = load(ones_d, [1, 2048])
        zeror = load(zeros_d, [1, 2048])
        iota = load(iota_d, [128, 40], I16)
        qarep_w = load(qarep_d, [16, 128])
        qbrep_w = load(qbrep_d, [16, 128])
        wd2aug = load(wd2aug_d, [18, 32])
        w1l = load(w1l_d, [48, 128])
        b1l = load(b1l_d, [128, 1])
        mw0 = load(mw0_d, [128, 64])
        mb0 = load(mb0_d, [64, 1])
        mw1 = load(mw1_d, [64, 64])
        mb1 = load(mb1_d, [64, 1])
        mw2 = load(mw2_d, [64, 1])
        mb2 = load(mb2_d, [1, 1])

        # persistent core-level tensors
        pooledT = wp.tile([128, G], F32)

        # pools
        pl_sc = P(name="scps", bufs=2, space="PSUM")      # [128,256] score psums
        pl_tb = P(name="tbps", bufs=2, space="PSUM")      # [128,256] table psums
        pl_ml = P(name="mlps", bufs=2, space="PSUM")      # [128,320] mlp psums
        pl_sm = P(name="smps", bufs=1, space="PSUM")      # small psums
        sc_p = P(name="scores", bufs=3)
        v_p = P(name="vals8", bufs=3)
        ix_p = P(name="idx", bufs=5)
        tb_p = P(name="tables", bufs=3)
        g_p = P(name="gath", bufs=3)
        h_p = P(name="hid", bufs=3)
        set_p = P(name="sets", bufs=2)
        s18_p = P(name="s18", bufs=2)
        f48_p = P(name="f48", bufs=2)

        # ---------------- global prep: sqG = -0.5*(x0^2+x1^2) [64,256] ----------
        xsq = sc_p.tile([128, 256], F32)
        nc.vector.tensor_tensor(out=xsq[:], in0=xtf[:], in1=xtf[:], op=ALU.mult)
        sq_ps = pl_tb.tile([64, 256], F32, tag="tbps")
        nc.tensor.matmul(sq_ps[:], lhsT=bdgf[:], rhs=xsq[:], start=True, stop=True)
        sqG = wp.tile([64, 256], F32)
        nc.scalar.copy(sqG[:], sq_ps[:])

        def topk_idx(scores_sb, ixt):
            """scores_sb [128,256] f32 (destroyed); returns dense [128,20] i16 of
            ranks 1..20 (rank 0 = self)."""
            for r in range(3):
                v = v_p.tile([128, 8], F32, tag="v8")
                nc.vector.max(v[:], scores_sb[:])
                nc.vector.max_index(ixt[:, 8 * r:8 * r + 8], v[:], scores_sb[:])
                if r < 2:
                    nc.vector.match_replace(scores_sb[:], v[:], scores_sb[:], NEG)
            ixd = ix_p.tile([128, 20], I16, tag="ixd")
            nc.vector.tensor_copy(out=ixd[:], in_=ixt[:, 1:21])
            return ixd

        def reduce_k_max(dst_ap, src_ap_320):
            # src [128, 320] (k-major: m = k*16+pb) -> max over k -> dst [128,16]
            v = src_ap_320.rearrange("p (k pb) -> p pb k", k=20, pb=16)
            nc.vector.tensor_reduce(out=dst_ap, in_=v, axis=AX.X, op=ALU.max)

        for s in range(sets):
            # ---------------- A4/B4 assembly ----------------
            a4 = set_p.tile([4, 2048], F32, tag="a4")
            nc.sync.dma_start(a4[:], a4_d.ap()[:, 2048 * s:2048 * (s + 1)])
            b4 = set_p.tile([4, 2048], F32, tag="b4")
            nc.sync.dma_start(b4[:], b4_d.ap()[:, 2048 * s:2048 * (s + 1)])
            nc.sync.dma_start(b4[2:3, :], sqG[8 * s:8 * s + 8, :])

            x1parts = set_p.tile([128, 256], F32, tag="x1p")   # [(ng,c), (g8,t,pb)]
            x1t8c = set_p.tile([16, 2048], F32, tag="x1c")     # [c, (g8,t,ng,pb)]

            # ---------------- conv1 per graph ----------------
            for g8 in range(8):
                gg = 8 * s + g8
                # kNN scores + selection, per 128-row half
                ixs = []
                for t in range(2):
                    ps = pl_sc.tile([128, 256], F32, tag="scps")
                    nc.tensor.matmul(
                        ps[:], lhsT=a4[:, 256 * g8 + 128 * t:256 * g8 + 128 * (t + 1)],
                        rhs=b4[:, 256 * g8:256 * (g8 + 1)], start=True, stop=True)
                    sc = sc_p.tile([128, 256], F32, tag="sc")
                    nc.scalar.copy(sc[:], ps[:])
                    ixt = ix_p.tile([128, 24], U16, tag="ix")
                    ixs.append(topk_idx(sc, ixt))
                # tables: CTrep / BTrep [128,256]
                pc = pl_tb.tile([128, 256], F32, tag="tbps")
                nc.tensor.matmul(pc[:], lhsT=cwrep[:],
                                 rhs=a4[0:2, 256 * g8:256 * (g8 + 1)],
                                 start=True, stop=True)
                ct = tb_p.tile([128, 256], F32, tag="ct")
                nc.scalar.add(ct[:], pc[:], b0rep[:])
                pb_ = pl_tb.tile([128, 256], F32, tag="tbps")
                nc.tensor.matmul(pb_[:], lhsT=bwrep[:],
                                 rhs=a4[0:2, 256 * g8:256 * (g8 + 1)],
                                 start=True, stop=True)
                bt = tb_p.tile([128, 256], F32, tag="bt")
                nc.scalar.copy(bt[:], pb_[:])
                for t in range(2):
                    cg = g_p.tile([128, 320], F32, tag="cg")
                    nc.gpsimd.ap_gather(cg[:], ct[:], iota[:, 20 * t:20 * t + 20],
                                        channels=128, num_elems=256, d=1, num_idxs=320)
                    bg = g_p.tile([128, 320], F32, tag="bg")
                    nc.gpsimd.ap_gather(bg[:], bt[:], ixs[t][:],
                                        channels=128, num_elems=256, d=1, num_idxs=320)
                    h1 = h_p.tile([128, 320], F32, tag="h1")
                    nc.gpsimd.tensor_tensor(out=h1[:], in0=bg[:], in1=cg[:], op=ALU.add)
                    nc.vector.tensor_scalar_max(out=h1[:], in0=h1[:], scalar1=0.0)
                    m1 = pl_ml.tile([128, 320], F32, tag="mlps")
                    nc.tensor.matmul(m1[:], lhsT=w1bd[:], rhs=h1[:], start=True, stop=True)
                    h2 = h_p.tile([128, 320], F32, tag="h2")
                    nc.scalar.activation(h2[:], m1[:], AF.Relu, bias=b1rep[:])
                    m2 = pl_ml.tile([128, 320], F32, tag="mlps")
                    nc.tensor.matmul(m2[:], lhsT=w2bd[:], rhs=h2[:], start=True, stop=True)
                    reduce_k_max(x1parts[:, 32 * g8 + 16 * t:32 * g8 + 16 * (t + 1)], m2[:])

            # x1 += b2 ; relayout to feature-major
            nc.vector.tensor_scalar_add(out=x1parts[:], in0=x1parts[:], scalar1=b2rep[:])
            x1v = x1t8c[:].rearrange("c (g t ng pb) -> c g t ng pb",
                                     g=8, t=2, ng=8, pb=16)
            for ng in range(8):
                # dst [16c, (g8,t,[ng],pb)] <- src [16c, (g8,t,pb)]
                nc.sync.dma_start(x1v[:, :, :, ng:ng + 1, :],
                                  x1parts[16 * ng:16 * ng + 16, :])

            # sq1 = -0.5 * sum_c x1^2  -> S18B row 16
            x1sq = set_p.tile([16, 2048], F32, tag="x1sq")
            nc.scalar.activation(x1sq[:], x1t8c[:], AF.Square)
            s18a = s18_p.tile([18, 2048], F32, tag="s18a")
            s18b = s18_p.tile([18, 2048], F32, tag="s18b")
            nc.sync.dma_start(s18a[0:16, :], x1t8c[:])
            nc.sync.dma_start(s18a[16:17, :], onesr[:])
            nc.sync.dma_start(s18a[17:18, :], zeror[:])
            nc.sync.dma_start(s18b[0:16, :], x1t8c[:])
            nc.sync.dma_start(s18b[17:18, :], onesr[:])
            sqrow = set_p.tile([1, 2048], F32, tag="sqrow")
            for q in range(4):
                mq = pl_sm.tile([1, 512], F32, tag="smps")
                nc.tensor.matmul(mq[:], lhsT=nh16[:], rhs=x1sq[:, 512 * q:512 * (q + 1)],
                                 start=True, stop=True)
                nc.scalar.copy(sqrow[:, 512 * q:512 * (q + 1)], mq[:])
            nc.sync.dma_start(s18b[16:17, :], sqrow[:])

            f48 = f48_p.tile([48, 2048], F32, tag="f48")
            nc.sync.dma_start(f48[0:16, :], x1t8c[:])

            x2pa = set_p.tile([128, 256], F32, tag="x2pa")
            x2pb = set_p.tile([128, 256], F32, tag="x2pb")

            # ---------------- conv2 per graph ----------------
            for g8 in range(8):
                ix2s = []
                for t in range(2):
                    ps = pl_sc.tile([128, 256], F32, tag="scps")
                    nc.tensor.matmul(
                        ps[:], lhsT=s18a[:, 256 * g8 + 128 * t:256 * g8 + 128 * (t + 1)],
                        rhs=s18b[:, 256 * g8:256 * (g8 + 1)], start=True, stop=True)
                    sc = sc_p.tile([128, 256], F32, tag="sc")
                    nc.scalar.copy(sc[:], ps[:])
                    ixt = ix_p.tile([128, 24], U16, tag="ix")
                    ix2s.append(topk_idx(sc, ixt))
                pq = pl_tb.tile([128, 256], F32, tag="tbps")
                nc.tensor.matmul(pq[:], lhsT=qarep_w[:],
                                 rhs=x1t8c[:, 256 * g8:256 * (g8 + 1)], start=True, stop=True)
                qa = tb_p.tile([128, 256], F32, tag="qa")
                nc.scalar.copy(qa[:], pq[:])
                pq2 = pl_tb.tile([128, 256], F32, tag="tbps")
                nc.tensor.matmul(pq2[:], lhsT=qbrep_w[:],
                                 rhs=x1t8c[:, 256 * g8:256 * (g8 + 1)], start=True, stop=True)
                qb = tb_p.tile([128, 256], F32, tag="qb")
                nc.scalar.copy(qb[:], pq2[:])
                for t in range(2):
                    ga = g_p.tile([128, 320], F32, tag="ga")
                    nc.gpsimd.ap_gather(ga[:], qa[:], ix2s[t][:],
                                        channels=128, num_elems=256, d=1, num_idxs=320)
                    reduce_k_max(x2pa[:, 32 * g8 + 16 * t:32 * g8 + 16 * (t + 1)], ga[:])
                    gb = g_p.tile([128, 320], F32, tag="gb")
                    nc.gpsimd.ap_gather(gb[:], qb[:], ix2s[t][:],
                                        channels=128, num_elems=256, d=1, num_idxs=320)
                    reduce_k_max(x2pb[:, 32 * g8 + 16 * t:32 * g8 + 16 * (t + 1)], gb[:])
                # linear term of conv2 (+bias) -> F48 rows 16..47
                l2 = pl_tb.tile([32, 256], F32, tag="tbps")
                nc.tensor.matmul(l2[:], lhsT=wd2aug[:],
                                 rhs=s18b[:, 256 * g8:256 * (g8 + 1)], start=True, stop=True)
                l2s = tb_p.tile([32, 256], F32, tag="l2s")
                nc.scalar.copy(l2s[:], l2[:])
                nc.sync.dma_start(f48[16:48, 256 * g8:256 * (g8 + 1)], l2s[:])

            # accumulate the aggregated max-terms into F48 rows 16..47
            f48v = f48[:].rearrange("c (g t ng pb) -> c g t ng pb",
                                    g=8, t=2, ng=8, pb=16)
            for ng in range(8):
                nc.gpsimd.dma_start(f48v[16:32, :, :, ng:ng + 1, :],
                                    x2pa[16 * ng:16 * ng + 16, :], accum_op=ALU.add)
                nc.gpsimd.dma_start(f48v[32:48, :, :, ng:ng + 1, :],
                                    x2pb[16 * ng:16 * ng + 16, :], accum_op=ALU.add)

            # ---------------- lin1 + global max pool ----------------
            for g8 in range(8):
                pl = pl_sc.tile([128, 256], F32, tag="scps")
                nc.tensor.matmul(pl[:], lhsT=w1l[:], rhs=f48[:, 256 * g8:256 * (g8 + 1)],
                                 start=True, stop=True)
                nc.vector.tensor_reduce(out=pooledT[:, 8 * s + g8:8 * s + g8 + 1],
                                        in_=pl[:], axis=AX.X, op=ALU.max)

        # ---------------- head MLP ----------------
        nc.vector.tensor_scalar_add(out=pooledT[:], in0=pooledT[:], scalar1=b1l[:])
        hd1 = pl_sm.tile([64, G], F32, tag="smps")
        nc.tensor.matmul(hd1[:], lhsT=mw0[:], rhs=pooledT[:], start=True, stop=True)
        h1s = wp.tile([64, G], F32)
        nc.scalar.activation(h1s[:], hd1[:], AF.Relu, bias=mb0[:])
        hd2 = pl_sm.tile([64, G], F32, tag="smps")
        nc.tensor.matmul(hd2[:], lhsT=mw1[:], rhs=h1s[:], start=True, stop=True)
        h2s = wp.tile([64, G], F32)
        nc.scalar.activation(h2s[:], hd2[:], AF.Relu, bias=mb1[:])
        hd3 = pl_sm.tile([1, G], F32, tag="smps")
        nc.tensor.matmul(hd3[:], lhsT=mw2[:], rhs=h2s[:], start=True, stop=True)
        outs = wp.tile([1, G], F32)
        nc.vector.tensor_scalar_add(out=outs[:], in0=hd3[:], scalar1=mb2[:])
        nc.sync.dma_start(out_d.ap(), outs[:])

    nc.compile()
    return nc


# ---------------------------------------------------------------------------
# Host-side input prep
# ---------------------------------------------------------------------------

def _tile8(w):
    return np.tile(np.asarray(w, np.float32), (1, 8) if w.ndim == 2 else 8)


def _blkdiag8(w):
    w = np.asarray(w, np.float32)
    out = np.zeros((128, 128), np.float32)
    for i in range(8):
        out[16 * i:16 * i + 16, 16 * i:16 * i + 16] = w
    return out


def make_in_maps(inputs):
    x = np.asarray(inputs["x"], np.float32)
    c1_w0 = np.asarray(inputs["c1_w0"], np.float32)
    consts = {}
    cw = c1_w0[:2] - c1_w0[2:4]           # [2,16]
    consts["cwrep"] = np.tile(cw, (1, 8)).astype(np.float32)                # [2,128]
    consts["bwrep"] = np.tile(c1_w0[2:4], (1, 8)).astype(np.float32)        # [2,128]
    consts["b0rep"] = np.tile(np.asarray(inputs["c1_b0"], np.float32), 8)[:, None]
    consts["w1bd"] = _blkdiag8(inputs["c1_w1"])
    consts["b1rep"] = np.tile(np.asarray(inputs["c1_b1"], np.float32), 8)[:, None]
    consts["w2bd"] = _blkdiag8(inputs["c1_w2"])
    consts["b2rep"] = np.tile(np.asarray(inputs["c1_b2"], np.float32), 8)[:, None]
    bdgf = np.zeros((128, 64), np.float32)
    for g in range(64):
        bdgf[2 * g, g] = -0.5
        bdgf[2 * g + 1, g] = -0.5
    consts["bdgf"] = bdgf
    consts["neghalf16"] = np.full((16, 1), -0.5, np.float32)
    consts["onesrow"] = np.ones((1, 2048), np.float32)
    consts["zerorow"] = np.zeros((1, 2048), np.float32)
    iota = np.zeros((128, 40), np.int16)
    for t in range(2):
        for p in range(128):
            iota[p, 20 * t:20 * t + 20] = 128 * t + p
    consts["iotaidx"] = iota
    c2_w0 = np.asarray(inputs["c2_w0"], np.float32)
    consts["wb2repA"] = np.tile(c2_w0[16:32, 0:16], (1, 8)).astype(np.float32)
    consts["wb2repB"] = np.tile(c2_w0[16:32, 16:32], (1, 8)).astype(np.float32)
    wd2aug = np.zeros((18, 32), np.float32)
    wd2aug[0:16] = c2_w0[0:16] - c2_w0[16:32]
    wd2aug[17] = np.asarray(inputs["c2_b0"], np.float32)
    consts["wd2aug"] = wd2aug
    consts["w1l"] = np.asarray(inputs["lin1_w"], np.float32)
    consts["b1l"] = np.asarray(inputs["lin1_b"], np.float32)[:, None]
    consts["mw0"] = np.asarray(inputs["m_w0"], np.float32)
    consts["mb0"] = np.asarray(inputs["m_b0"], np.float32)[:, None]
    consts["mw1"] = np.asarray(inputs["m_w1"], np.float32)
    consts["mb1"] = np.asarray(inputs["m_b1"], np.float32)[:, None]
    consts["mw2"] = np.asarray(inputs["m_w2"], np.float32)
    consts["mb2"] = np.asarray(inputs["m_b2"], np.float32)[:, None]

    in_maps = []
    npc = N * GPC  # nodes per core
    for c in range(NCORES):
        xc = x[c * npc:(c + 1) * npc]                       # [16384, 2]
        xg = xc.reshape(GPC, N, 2)
        m = dict(consts)
        m["xtf"] = xg.transpose(0, 2, 1).reshape(128, 256).copy()   # [2g+f, j]
        rows = xc.T.reshape(2, -1)                          # [f, 256g+j]
        a4 = np.zeros((4, 16384), np.float32)
        a4[0:2] = rows
        a4[2] = 1.0
        m["a4all"] = a4
        b4 = np.zeros((4, 16384), np.float32)
        b4[0:2] = rows
        b4[3] = 1.0
        m["b4all"] = b4
        in_maps.append(m)
    return in_maps


_CACHED = {}


def _get_program(sets=8):
    if sets not in _CACHED:
        _CACHED[sets] = build_program(sets)
    return _CACHED[sets]


def run(inputs, trace=False, **kw):
    nc = _get_program(8)
    in_maps = make_in_maps(inputs)
    res = run_bass_kernel_spmd(nc, in_maps, list(range(NCORES)), trace=trace, **kw)
    out = np.concatenate([res.results[c]["out"].reshape(GPC) for c in range(NCORES)])
    return out.reshape(B, 1).astype(np.float32), res


def kernel(**inputs) -> np.ndarray:
    out, _ = run(inputs, trace=False)
    return out



# revision 49
# speedup vs baseline: 1.4303x; 1.4303x over previous
"""DGCNN-style GNN (2x dynamic-kNN EdgeConv + global pool + MLP head) on 8 Trainium2
NeuronCores, data-parallel over the 512 graphs (64 graphs per core).

Self-contained: hardcodes all shapes; host side only does layout prep (transpose /
tiling / block-diag packing of weights) and sharding.

v2: GpSimd runs only ap_gather (no lib swaps); the center-node term of conv1 is
applied as a broadcast-view vector add instead of an iota gather; conv2's
aggregated terms land in f48 via plain relayout DMAs + one vector add instead of
accumulate-DMAs; non-score matmuls run in float32r (1 cycle/row vs fp32's 4).
"""

import sys

for _p in ("/opt/trn_rl_repo",):
    if _p not in sys.path:
        sys.path.append(_p)

from contextlib import ExitStack

import numpy as np

import concourse.bass as bass
import concourse.tile as tile
from concourse import bacc, mybir
from concourse.bass_utils import run_bass_kernel_spmd

F32 = mybir.dt.float32
F32R = mybir.dt.float32r
U16 = mybir.dt.uint16
I16 = mybir.dt.int16
AF = mybir.ActivationFunctionType
ALU = mybir.AluOpType
AX = mybir.AxisListType

B, N, K = 512, 256, 20
NCORES = 8
GPC = B // NCORES  # graphs per core = 64
NEG = -1.0e30

USE_F32R = True


# ---------------------------------------------------------------------------
# Bass program builder (identical program for every core; all data via inputs)
# ---------------------------------------------------------------------------

def build_program(sets: int = 8):
    """Build the per-core Bass program. `sets` = number of 8-graph sets (8 => 64
    graphs). Returns the compiled Bacc object."""
    G = 8 * sets
    nc = bacc.Bacc("TRN2", target_bir_lowering=False, debug=False)

    def din(name, shape, dtype=F32):
        return nc.declare_dram_parameter(name, list(shape), dtype, isOutput=False)

    # -------------------- DRAM parameters --------------------
    xtf_d = din("xtf", [128, 256])          # [2g+f, j] = x[256g+j, f] (g<64)
    a4_d = din("a4all", [4, 16384])         # rows: x0, x1, ones, zeros
    b4_d = din("b4all", [4, 16384])         # rows: x0, x1, zeros(->-sq/2), ones
    cw2_d = din("cw2", [2, 16])             # c1_w0[:2] - c1_w0[2:4]
    b0c_d = din("b0c", [16, 1])             # c1_b0
    bwrep_d = din("bwrep", [2, 128])        # tile8(c1_w0[2:4])
    w1bd_d = din("w1bd", [128, 128])        # blkdiag8(c1_w1)
    b1rep_d = din("b1rep", [128, 1])
    w2bd_d = din("w2bd", [128, 128])        # blkdiag8(c1_w2)
    b2rep_d = din("b2rep", [128, 1])
    bdgf_d = din("bdgf", [128, 64])         # [2g+f, g] = -0.5
    nh16_d = din("neghalf16", [16, 1])      # -0.5
    qarep_d = din("wb2repA", [16, 128])     # tile8(c2_w0[16:32, :16])
    qbrep_d = din("wb2repB", [16, 128])     # tile8(c2_w0[16:32, 16:])
    wd2aug_d = din("wd2aug", [18, 32])      # [c2_w0[:16]-c2_w0[16:32]; 0; c2_b0]
    w1l_d = din("w1l", [48, 128])           # lin1_w
    b1l_d = din("b1l", [128, 1])            # lin1_b
    mw0_d = din("mw0", [128, 64])
    mb0_d = din("mb0", [64, 1])
    mw1_d = din("mw1", [64, 64])
    mb1_d = din("mb1", [64, 1])
    mw2_d = din("mw2", [64, 1])
    mb2_d = din("mb2", [1, 1])
    out_d = nc.declare_dram_parameter("out", [1, G], F32, isOutput=True)

    with tile.TileContext(nc) as tc, ExitStack() as ctx:
        P = lambda **kw: ctx.enter_context(tc.tile_pool(**kw))
        wp = P(name="weights", bufs=1)

        def load(dram, shape, dtype=F32):
            t = wp.tile(list(shape), dtype, tag=dram.name)
            src = dram.ap()
            if dtype == F32R:
                src = src.bitcast(F32R)
            nc.sync.dma_start(t[:], src)
            return t

        xtf = load(xtf_d, [128, 256])
        cw2 = load(cw2_d, [2, 16], F32R)
        b0c = load(b0c_d, [16, 1])
        bwrep = load(bwrep_d, [2, 128], F32R)
        w1bd = load(w1bd_d, [128, 128], F32R)
        b1rep = load(b1rep_d, [128, 1])
        w2bd = load(w2bd_d, [128, 128], F32R)
        b2rep = load(b2rep_d, [128, 1])
        bdgf = load(bdgf_d, [128, 64])
        nh16 = load(nh16_d, [16, 1], F32R)
        qarep_w = load(qarep_d, [16, 128], F32R)
        qbrep_w = load(qbrep_d, [16, 128], F32R)
        wd2aug = load(wd2aug_d, [18, 32])
        w1l = load(w1l_d, [48, 128], F32R)
        b1l = load(b1l_d, [128, 1])
        mw0 = load(mw0_d, [128, 64])
        mb0 = load(mb0_d, [64, 1])
        mw1 = load(mw1_d, [64, 64])
        mb1 = load(mb1_d, [64, 1])
        mw2 = load(mw2_d, [64, 1])
        mb2 = load(mb2_d, [1, 1])

        # persistent core-level tensors
        pooledT = wp.tile([128, G], F32)

        # pools
        pl_sc = P(name="scps", bufs=2, space="PSUM")      # [128,256] score psums
        pl_tb = P(name="tbps", bufs=2, space="PSUM")      # [128,256] table psums
        pl_ml = P(name="mlps", bufs=2, space="PSUM")      # [128,320] mlp psums
        pl_sm = P(name="smps", bufs=1, space="PSUM")      # small psums
        pl_ct = P(name="ctps", bufs=1, space="PSUM")      # [16,512] ctall psums
        sc_p = P(name="scores", bufs=3)
        v_p = P(name="vals8", bufs=3)
        ix_p = P(name="idx", bufs=5)
        tb_p = P(name="tables", bufs=3)
        g_p = P(name="gath", bufs=3)
        h_p = P(name="hid", bufs=3)
        set_p = P(name="sets", bufs=2)
        s18_p = P(name="s18", bufs=2)
        f48_p = P(name="f48", bufs=2)
        big1_p = P(name="big1", bufs=1)

        # ---------------- global prep: sqG = -0.5*(x0^2+x1^2) [64,256] ----------
        xsq = sc_p.tile([128, 256], F32)
        nc.vector.tensor_tensor(out=xsq[:], in0=xtf[:], in1=xtf[:], op=ALU.mult)
        sq_ps = pl_tb.tile([64, 256], F32, tag="tbps")
        nc.tensor.matmul(sq_ps[:], lhsT=bdgf[:], rhs=xsq[:], start=True, stop=True)
        sqG = wp.tile([64, 256], F32)
        nc.scalar.copy(sqG[:], sq_ps[:])

        def topk_idx(scores_sb, ixt):
            """scores_sb [128,256] f32 (destroyed); returns dense [128,20] i16 of
            ranks 1..20 (rank 0 = self)."""
            for r in range(3):
                v = v_p.tile([128, 8], F32, tag="v8")
                nc.vector.max(v[:], scores_sb[:])
                nc.vector.max_index(ixt[:, 8 * r:8 * r + 8], v[:], scores_sb[:])
                if r < 2:
                    nc.vector.match_replace(scores_sb[:], v[:], scores_sb[:], NEG)
            ixd = ix_p.tile([128, 20], I16, tag="ixd")
            nc.vector.tensor_copy(out=ixd[:], in_=ixt[:, 1:21])
            return ixd

        def reduce_k_max(dst_ap, src_ap_320):
            # src [128, 320] (k-major: m = k*16+pb) -> max over k -> dst [128,16]
            v = src_ap_320.rearrange("p (k pb) -> p pb k", k=20, pb=16)
            nc.vector.tensor_reduce(out=dst_ap, in_=v, axis=AX.X, op=ALU.max)

        for s in range(sets):
            # ---------------- A4/B4 assembly ----------------
            a4 = set_p.tile([4, 2048], F32R, tag="a4")
            nc.sync.dma_start(a4[:], a4_d.ap()[:, 2048 * s:2048 * (s + 1)].bitcast(F32R))
            b4 = set_p.tile([4, 2048], F32, tag="b4")
            nc.sync.dma_start(b4[:], b4_d.ap()[:, 2048 * s:2048 * (s + 1)])
            nc.sync.dma_start(b4[2:3, :], sqG[8 * s:8 * s + 8, :])

            # ---------------- conv1 center-term table (per set) ----------------
            # ctall [16c, (g t ng pb)] = (c1_w0[:2]-c1_w0[2:4])^T x + b0, then
            # relayout to ct2set [(ng,c), (g,t,pb)] (per-node center term).
            ctall = big1_p.tile([16, 2048], F32, tag="ctall")
            for q in range(4):
                cps = pl_ct.tile([16, 512], F32, tag="ctps")
                nc.tensor.matmul(cps[:], lhsT=cw2[:],
                                 rhs=a4[0:2, 512 * q:512 * (q + 1)],
                                 start=True, stop=True)
                nc.scalar.add(ctall[:, 512 * q:512 * (q + 1)], cps[:], b0c[:])
            ct2set = set_p.tile([128, 256], F32, tag="ct2")
            ctv = ctall[:].rearrange("c (g t ng pb) -> c g t ng pb",
                                     g=8, t=2, ng=8, pb=16)
            for ng in range(8):
                nc.sync.dma_start(ct2set[16 * ng:16 * ng + 16, :],
                                  ctv[:, :, :, ng, :])

            x1parts = set_p.tile([128, 256], F32, tag="x1p")   # [(ng,c), (g8,t,pb)]
            x1t8c = set_p.tile([16, 2048], F32R, tag="x1c")    # [c, (g8,t,ng,pb)]

            # ---------------- conv1 per graph ----------------
            for g8 in range(8):
                gg = 8 * s + g8
                # kNN scores + selection, per 128-row half
                ixs = []
                for t in range(2):
                    ps = pl_sc.tile([128, 256], F32, tag="scps")
                    nc.tensor.matmul(
                        ps[:],
                        lhsT=a4[:, 256 * g8 + 128 * t:256 * g8 + 128 * (t + 1)].bitcast(F32),
                        rhs=b4[:, 256 * g8:256 * (g8 + 1)], start=True, stop=True)
                    sc = sc_p.tile([128, 256], F32, tag="sc")
                    nc.scalar.copy(sc[:], ps[:])
                    ixt = ix_p.tile([128, 24], U16, tag="ix")
                    ixs.append(topk_idx(sc, ixt))
                # neighbor-term table BTrep [128,256]
                pb_ = pl_tb.tile([128, 256], F32, tag="tbps")
                nc.tensor.matmul(pb_[:], lhsT=bwrep[:],
                                 rhs=a4[0:2, 256 * g8:256 * (g8 + 1)],
                                 start=True, stop=True)
                bt = tb_p.tile([128, 256], F32, tag="bt")
                nc.scalar.copy(bt[:], pb_[:])
                for t in range(2):
                    bg = g_p.tile([128, 320], F32, tag="bg")
                    nc.gpsimd.ap_gather(bg[:], bt[:], ixs[t][:],
                                        channels=128, num_elems=256, d=1, num_idxs=320)
                    # h1 = bg + center-term (broadcast over k)
                    ctb = ct2set[:, 32 * g8 + 16 * t:32 * g8 + 16 * (t + 1)]
                    ctb = ctb.unsqueeze(1).broadcast_to((128, 20, 16))
                    h1 = h_p.tile([128, 320], F32R, tag="h1")
                    h1v = h1[:].rearrange("p (k pb) -> p k pb", k=20, pb=16)
                    bgv = bg[:].rearrange("p (k pb) -> p k pb", k=20, pb=16)
                    nc.vector.tensor_tensor(out=h1v, in0=bgv, in1=ctb, op=ALU.add)
                    nc.vector.tensor_scalar_max(out=h1[:], in0=h1[:], scalar1=0.0)
                    m1 = pl_ml.tile([128, 320], F32, tag="mlps")
                    nc.tensor.matmul(m1[:], lhsT=w1bd[:], rhs=h1[:],
                                     start=True, stop=True)
                    h2 = h_p.tile([128, 320], F32R, tag="h2")
                    nc.scalar.activation(h2[:], m1[:], AF.Relu, bias=b1rep[:])
                    m2 = pl_ml.tile([128, 320], F32, tag="mlps")
                    nc.tensor.matmul(m2[:], lhsT=w2bd[:], rhs=h2[:],
                                     start=True, stop=True)
                    reduce_k_max(x1parts[:, 32 * g8 + 16 * t:32 * g8 + 16 * (t + 1)], m2[:])

            # x1 += b2 ; relayout to feature-major
            nc.vector.tensor_scalar_add(out=x1parts[:], in0=x1parts[:], scalar1=b2rep[:])
            x1v = x1t8c[:].rearrange("c (g t ng pb) -> c g t ng pb",
                                     g=8, t=2, ng=8, pb=16)
            for ng in range(8):
                # dst [16c, (g8,t,[ng],pb)] <- src [16c, (g8,t,pb)]
                nc.sync.dma_start(x1v[:, :, :, ng:ng + 1, :],
                                  x1parts[16 * ng:16 * ng + 16, :].bitcast(F32R))

            # sq1 = -0.5 * sum_c x1^2  -> S18B row 16
            x1sq = set_p.tile([16, 2048], F32R, tag="x1sq")
            nc.scalar.activation(x1sq[:], x1t8c[:], AF.Square)
            s18a = s18_p.tile([18, 2048], F32, tag="s18a")
            s18b = s18_p.tile([18, 2048], F32, tag="s18b")
            # row layout: s18a = [ones; x1(16); zeros], s18b = [sq; x1(16); ones]
            nc.sync.dma_start(s18a[1:17, :], x1t8c[:].bitcast(F32))
            nc.sync.dma_start(s18a[0:1, :], b4[3:4, :])   # ones
            nc.sync.dma_start(s18a[17:18, :], a4[3:4, :].bitcast(F32))   # zeros
            nc.sync.dma_start(s18b[1:17, :], x1t8c[:].bitcast(F32))
            nc.sync.dma_start(s18b[17:18, :], b4[3:4, :])   # ones
            for q in range(4):
                mq = pl_sm.tile([1, 512], F32, tag="smps")
                nc.tensor.matmul(mq[:], lhsT=nh16[:], rhs=x1sq[:, 512 * q:512 * (q + 1)],
                                 start=True, stop=True)
                nc.scalar.copy(s18b[0:1, 512 * q:512 * (q + 1)], mq[:])

            # f48 rows: 0:32 = conv2 features, 32:48 = x1 (w1l rows permuted to match)
            f48 = f48_p.tile([48, 2048], F32R, tag="f48")
            nc.sync.dma_start(f48[32:48, :], x1t8c[:])

            x2pa = set_p.tile([128, 256], F32, tag="x2pa")
            x2pb = set_p.tile([128, 256], F32, tag="x2pb")

            # ---------------- conv2 per graph ----------------
            for g8 in range(8):
                ix2s = []
                for t in range(2):
                    ps = pl_sc.tile([128, 256], F32, tag="scps")
                    nc.tensor.matmul(
                        ps[:], lhsT=s18a[:, 256 * g8 + 128 * t:256 * g8 + 128 * (t + 1)],
                        rhs=s18b[:, 256 * g8:256 * (g8 + 1)], start=True, stop=True)
                    sc = sc_p.tile([128, 256], F32, tag="sc")
                    nc.scalar.copy(sc[:], ps[:])
                    ixt = ix_p.tile([128, 24], U16, tag="ix")
                    ix2s.append(topk_idx(sc, ixt))
                pq = pl_tb.tile([128, 256], F32, tag="tbps")
                nc.tensor.matmul(pq[:], lhsT=qarep_w[:],
                                 rhs=x1t8c[:, 256 * g8:256 * (g8 + 1)], start=True, stop=True)
                qa = tb_p.tile([128, 256], F32, tag="qa")
                nc.scalar.copy(qa[:], pq[:])
                pq2 = pl_tb.tile([128, 256], F32, tag="tbps")
                nc.tensor.matmul(pq2[:], lhsT=qbrep_w[:],
                                 rhs=x1t8c[:, 256 * g8:256 * (g8 + 1)], start=True, stop=True)
                qb = tb_p.tile([128, 256], F32, tag="qb")
                nc.scalar.copy(qb[:], pq2[:])
                for t in range(2):
                    ga = g_p.tile([128, 320], F32, tag="ga")
                    nc.gpsimd.ap_gather(ga[:], qa[:], ix2s[t][:],
                                        channels=128, num_elems=256, d=1, num_idxs=320)
                    reduce_k_max(x2pa[:, 32 * g8 + 16 * t:32 * g8 + 16 * (t + 1)], ga[:])
                    gb = g_p.tile([128, 320], F32, tag="gb")
                    nc.gpsimd.ap_gather(gb[:], qb[:], ix2s[t][:],
                                        channels=128, num_elems=256, d=1, num_idxs=320)
                    reduce_k_max(x2pb[:, 32 * g8 + 16 * t:32 * g8 + 16 * (t + 1)], gb[:])
                # linear term of conv2 (+bias) -> F48 rows 16..47 directly
                l2 = pl_tb.tile([32, 256], F32, tag="tbps")
                nc.tensor.matmul(l2[:], lhsT=wd2aug[:],
                                 rhs=s18b[:, 256 * g8:256 * (g8 + 1)], start=True, stop=True)
                nc.scalar.copy(f48[0:32, 256 * g8:256 * (g8 + 1)], l2[:])

            # relayout aggregated max-terms, then one add into F48 rows 16..47
            x2t = big1_p.tile([32, 2048], F32, tag="x2t")
            x2tv = x2t[:].rearrange("c (g t ng pb) -> c g t ng pb",
                                    g=8, t=2, ng=8, pb=16)
            for ng in range(8):
                nc.sync.dma_start(x2tv[0:16, :, :, ng:ng + 1, :],
                                  x2pa[16 * ng:16 * ng + 16, :])
                nc.sync.dma_start(x2tv[16:32, :, :, ng:ng + 1, :],
                                  x2pb[16 * ng:16 * ng + 16, :])
            nc.vector.tensor_tensor(out=f48[0:32, :], in0=f48[0:32, :],
                                    in1=x2t[:], op=ALU.add)

            # ---------------- lin1 + global max pool ----------------
            for g8 in range(8):
                pl = pl_sc.tile([128, 256], F32, tag="scps")
                nc.tensor.matmul(pl[:], lhsT=w1l[:], rhs=f48[:, 256 * g8:256 * (g8 + 1)],
                                 start=True, stop=True)
                nc.vector.tensor_reduce(out=pooledT[:, 8 * s + g8:8 * s + g8 + 1],
                                        in_=pl[:], axis=AX.X, op=ALU.max)

        # ---------------- head MLP ----------------
        nc.vector.tensor_scalar_add(out=pooledT[:], in0=pooledT[:], scalar1=b1l[:])
        hd1 = pl_sm.tile([64, G], F32, tag="smps")
        nc.tensor.matmul(hd1[:], lhsT=mw0[:], rhs=pooledT[:], start=True, stop=True)
        h1s = wp.tile([64, G], F32)
        nc.scalar.activation(h1s[:], hd1[:], AF.Relu, bias=mb0[:])
        hd2 = pl_sm.tile([64, G], F32, tag="smps")
        nc.tensor.matmul(hd2[:], lhsT=mw1[:], rhs=h1s[:], start=True, stop=True)
        h2s = wp.tile([64, G], F32)
        nc.scalar.activation(h2s[:], hd2[:], AF.Relu, bias=mb1[:])
        hd3 = pl_sm.tile([1, G], F32, tag="smps")
        nc.tensor.matmul(hd3[:], lhsT=mw2[:], rhs=h2s[:], start=True, stop=True)
        outs = wp.tile([1, G], F32)
        nc.vector.tensor_scalar_add(out=outs[:], in0=hd3[:], scalar1=mb2[:])
        nc.sync.dma_start(out_d.ap(), outs[:])

    nc.compile()
    return nc


# ---------------------------------------------------------------------------
# Host-side input prep
# ---------------------------------------------------------------------------

def _blkdiag8(w):
    w = np.asarray(w, np.float32)
    out = np.zeros((128, 128), np.float32)
    for i in range(8):
        out[16 * i:16 * i + 16, 16 * i:16 * i + 16] = w
    return out


def make_in_maps(inputs):
    x = np.asarray(inputs["x"], np.float32)
    c1_w0 = np.asarray(inputs["c1_w0"], np.float32)
    consts = {}
    consts["cw2"] = (c1_w0[:2] - c1_w0[2:4]).astype(np.float32)              # [2,16]
    consts["b0c"] = np.asarray(inputs["c1_b0"], np.float32)[:, None]         # [16,1]
    consts["bwrep"] = np.tile(c1_w0[2:4], (1, 8)).astype(np.float32)         # [2,128]
    consts["w1bd"] = _blkdiag8(inputs["c1_w1"])
    consts["b1rep"] = np.tile(np.asarray(inputs["c1_b1"], np.float32), 8)[:, None]
    consts["w2bd"] = _blkdiag8(inputs["c1_w2"])
    consts["b2rep"] = np.tile(np.asarray(inputs["c1_b2"], np.float32), 8)[:, None]
    bdgf = np.zeros((128, 64), np.float32)
    for g in range(64):
        bdgf[2 * g, g] = -0.5
        bdgf[2 * g + 1, g] = -0.5
    consts["bdgf"] = bdgf
    consts["neghalf16"] = np.full((16, 1), -0.5, np.float32)
    c2_w0 = np.asarray(inputs["c2_w0"], np.float32)
    consts["wb2repA"] = np.tile(c2_w0[16:32, 0:16], (1, 8)).astype(np.float32)
    consts["wb2repB"] = np.tile(c2_w0[16:32, 16:32], (1, 8)).astype(np.float32)
    # rows match s18b = [sq; x1(16); ones]
    wd2aug = np.zeros((18, 32), np.float32)
    wd2aug[1:17] = c2_w0[0:16] - c2_w0[16:32]
    wd2aug[17] = np.asarray(inputs["c2_b0"], np.float32)
    consts["wd2aug"] = wd2aug
    w1l = np.asarray(inputs["lin1_w"], np.float32)
    consts["w1l"] = np.concatenate([w1l[16:48], w1l[0:16]], axis=0)  # f48 row order
    consts["b1l"] = np.asarray(inputs["lin1_b"], np.float32)[:, None]
    consts["mw0"] = np.asarray(inputs["m_w0"], np.float32)
    consts["mb0"] = np.asarray(inputs["m_b0"], np.float32)[:, None]
    consts["mw1"] = np.asarray(inputs["m_w1"], np.float32)
    consts["mb1"] = np.asarray(inputs["m_b1"], np.float32)[:, None]
    consts["mw2"] = np.asarray(inputs["m_w2"], np.float32)
    consts["mb2"] = np.asarray(inputs["m_b2"], np.float32)[:, None]

    in_maps = []
    npc = N * GPC  # nodes per core
    for c in range(NCORES):
        xc = x[c * npc:(c + 1) * npc]                       # [16384, 2]
        xg = xc.reshape(GPC, N, 2)
        m = dict(consts)
        m["xtf"] = xg.transpose(0, 2, 1).reshape(128, 256).copy()   # [2g+f, j]
        rows = xc.T.reshape(2, -1)                          # [f, 256g+j]
        a4 = np.zeros((4, 16384), np.float32)
        a4[0:2] = rows
        a4[2] = 1.0
        m["a4all"] = a4
        b4 = np.zeros((4, 16384), np.float32)
        b4[0:2] = rows
        b4[3] = 1.0
        m["b4all"] = b4
        in_maps.append(m)
    return in_maps


_CACHED = {}


def _get_program(sets=8):
    if sets not in _CACHED:
        _CACHED[sets] = build_program(sets)
    return _CACHED[sets]


def run(inputs, trace=False, **kw):
    nc = _get_program(8)
    in_maps = make_in_maps(inputs)
    res = run_bass_kernel_spmd(nc, in_maps, list(range(NCORES)), trace=trace, **kw)
    out = np.concatenate([res.results[c]["out"].reshape(GPC) for c in range(NCORES)])
    return out.reshape(B, 1).astype(np.float32), res


def kernel(**inputs) -> np.ndarray:
    out, _ = run(inputs, trace=False)
    return out


# revision 54
# speedup vs baseline: 1.7646x; 1.2337x over previous
"""DGCNN-style GNN (2x dynamic-kNN EdgeConv + global pool + MLP head) on 8 Trainium2
NeuronCores, data-parallel over the 512 graphs (64 graphs per core).

Self-contained: hardcodes all shapes; host side only does layout prep (transpose /
tiling / block-diag packing of weights) and sharding.

v3: ap_gather has ~8.5us dispatch overhead per call, so gathers are batched at the
set level (one gather over an 8-graph 2048-node table with offset indices): 16
gather calls total instead of 384. conv2's two neighbor tables are interleaved
(d=2, bf16) into a single gather. kNN topk runs on bf16 scores (2x DVE). Non-score
matmuls run in float32r (1 cycle/row vs fp32's 4).
"""

import sys

for _p in ("/opt/trn_rl_repo",):
    if _p not in sys.path:
        sys.path.append(_p)

from contextlib import ExitStack

import numpy as np

import concourse.bass as bass
import concourse.tile as tile
from concourse import bacc, mybir
from concourse.bass_utils import run_bass_kernel_spmd

F32 = mybir.dt.float32
F32R = mybir.dt.float32r
BF16 = mybir.dt.bfloat16
U16 = mybir.dt.uint16
I16 = mybir.dt.int16
AF = mybir.ActivationFunctionType
ALU = mybir.AluOpType
AX = mybir.AxisListType

B, N, K = 512, 256, 20
NCORES = 8
GPC = B // NCORES  # graphs per core = 64
NEG = -1.0e30


# ---------------------------------------------------------------------------
# Bass program builder (identical program for every core; all data via inputs)
# ---------------------------------------------------------------------------

def build_program(sets: int = 8):
    """Build the per-core Bass program. `sets` = number of 8-graph sets (8 => 64
    graphs). Returns the compiled Bacc object."""
    G = 8 * sets
    nc = bacc.Bacc("TRN2", target_bir_lowering=False, debug=False)

    def din(name, shape, dtype=F32):
        return nc.declare_dram_parameter(name, list(shape), dtype, isOutput=False)

    # -------------------- DRAM parameters --------------------
    xtf_d = din("xtf", [128, 256])          # [2g+f, j] = x[256g+j, f] (g<64)
    a4_d = din("a4all", [4, 16384])         # rows: x0, x1, ones, zeros
    b4_d = din("b4all", [4, 16384])         # rows: x0, x1, zeros(->-sq/2), ones
    cw2_d = din("cw2", [2, 16])             # c1_w0[:2] - c1_w0[2:4]
    b0c_d = din("b0c", [16, 1])             # c1_b0
    bwrep_d = din("bwrep", [2, 128])        # tile8(c1_w0[2:4])
    w1bd_d = din("w1bd", [128, 128])        # blkdiag8(c1_w1)
    b1rep_d = din("b1rep", [128, 1])
    w2bd_d = din("w2bd", [128, 128])        # blkdiag8(c1_w2)
    b2rep_d = din("b2rep", [128, 1])
    bdgf_d = din("bdgf", [128, 64])         # [2g+f, g] = -0.5
    nh16_d = din("neghalf16", [16, 1])      # -0.5
    goffs_d = din("goffs", [128, 320], I16)  # [:, 40g+j] = 256*g
    qarep_d = din("wb2repA", [16, 128])     # tile8(c2_w0[16:32, :16])
    qbrep_d = din("wb2repB", [16, 128])     # tile8(c2_w0[16:32, 16:])
    wd2aug_d = din("wd2aug", [18, 32])      # rows match s18b = [sq; x1; ones]
    w1l_d = din("w1l", [48, 128])           # lin1_w, rows = [x2(32); x1(16)]
    b1l_d = din("b1l", [128, 1])            # lin1_b
    mw0_d = din("mw0", [128, 64])
    mb0_d = din("mb0", [64, 1])
    mw1_d = din("mw1", [64, 64])
    mb1_d = din("mb1", [64, 1])
    mw2_d = din("mw2", [64, 1])
    mb2_d = din("mb2", [1, 1])
    out_d = nc.declare_dram_parameter("out", [1, G], F32, isOutput=True)

    with tile.TileContext(nc) as tc, ExitStack() as ctx:
        P = lambda **kw: ctx.enter_context(tc.tile_pool(**kw))
        wp = P(name="weights", bufs=1)

        def load(dram, shape, dtype=F32):
            t = wp.tile(list(shape), dtype, tag=dram.name)
            src = dram.ap()
            if dtype == F32R:
                src = src.bitcast(F32R)
            nc.sync.dma_start(t[:], src)
            return t

        xtf = load(xtf_d, [128, 256])
        cw2 = load(cw2_d, [2, 16], F32R)
        b0c = load(b0c_d, [16, 1])
        bwrep = load(bwrep_d, [2, 128], F32R)
        w1bd = load(w1bd_d, [128, 128], F32R)
        b1rep = load(b1rep_d, [128, 1])
        w2bd = load(w2bd_d, [128, 128], F32R)
        b2rep = load(b2rep_d, [128, 1])
        bdgf = load(bdgf_d, [128, 64])
        nh16 = load(nh16_d, [16, 1], F32R)
        goffs = load(goffs_d, [128, 320], I16)
        qarep_w = load(qarep_d, [16, 128], F32R)
        qbrep_w = load(qbrep_d, [16, 128], F32R)
        wd2aug = load(wd2aug_d, [18, 32])
        w1l = load(w1l_d, [48, 128], F32R)
        b1l = load(b1l_d, [128, 1])
        mw0 = load(mw0_d, [128, 64])
        mb0 = load(mb0_d, [64, 1])
        mw1 = load(mw1_d, [64, 64])
        mb1 = load(mb1_d, [64, 1])
        mw2 = load(mw2_d, [64, 1])
        mb2 = load(mb2_d, [1, 1])

        # persistent core-level tensors
        pooledT = wp.tile([128, G], F32)

        # PSUM pools: 2 + 2 + 4 banks = 8
        pl_sc = P(name="scps", bufs=2, space="PSUM")      # [128,256] score psums
        pl_ml = P(name="mlps", bufs=2, space="PSUM")      # [128,320] mlp psums
        pl_bg = P(name="bgps", bufs=4, space="PSUM")      # [128,512] everything else

        sc_p = P(name="scores", bufs=3)
        v_p = P(name="vals8", bufs=3)
        ix_p = P(name="idx", bufs=4)
        h_p = P(name="hid", bufs=3)
        set_p = P(name="sets", bufs=2)
        s18_p = P(name="s18", bufs=1)
        f48_p = P(name="f48", bufs=2)
        big1_p = P(name="big1", bufs=1)

        # ---------------- global prep: sqG = -0.5*(x0^2+x1^2) [64,256] ----------
        xsq = sc_p.tile([128, 256], F32, tag="xsq")
        nc.vector.tensor_tensor(out=xsq[:], in0=xtf[:], in1=xtf[:], op=ALU.mult)
        sq_ps = pl_bg.tile([128, 512], F32, tag="bgps")
        nc.tensor.matmul(sq_ps[0:64, 0:256], lhsT=bdgf[:], rhs=xsq[:], start=True, stop=True)
        sqG = wp.tile([64, 256], F32)
        nc.scalar.copy(sqG[:], sq_ps[0:64, 0:256])

        def topk20(scores_sb, ixset, col0):
            """scores_sb [128,256] bf16 (destroyed); writes node indices of ranks
            1..20 (rank 0 = self) into ixset[:, col0:col0+20] as i16."""
            ixt = ix_p.tile([128, 24], U16, tag="ix")
            for r in range(3):
                v = v_p.tile([128, 8], BF16, tag="v8")
                nc.vector.max(v[:], scores_sb[:])
                nc.vector.max_index(ixt[:, 8 * r:8 * r + 8], v[:], scores_sb[:])
                if r < 2:
                    nc.vector.match_replace(scores_sb[:], v[:], scores_sb[:], NEG)
            nc.vector.tensor_copy(out=ixset[:, col0:col0 + 20], in_=ixt[:, 1:21])

        for s in range(sets):
            # ---------------- A4/B4 assembly ----------------
            a4 = set_p.tile([4, 2048], F32R, tag="a4")
            nc.sync.dma_start(a4[:], a4_d.ap()[:, 2048 * s:2048 * (s + 1)].bitcast(F32R))
            b4 = set_p.tile([4, 2048], F32, tag="b4")
            nc.sync.dma_start(b4[:], b4_d.ap()[:, 2048 * s:2048 * (s + 1)])
            nc.sync.dma_start(b4[2:3, :], sqG[8 * s:8 * s + 8, :])

            # ---------------- conv1 tables (per set) ----------------
            # ctall [16c, (g t ng pb)] = (c1_w0[:2]-c1_w0[2:4])^T x + b0, then
            # relayout to ct2set [(ng,c), (g,t,pb)] (per-node center term).
            ctall = big1_p.tile([16, 2048], F32, tag="ctall")
            for q in range(4):
                cps = pl_bg.tile([128, 512], F32, tag="bgps")
                nc.tensor.matmul(cps[0:16, :], lhsT=cw2[:],
                                 rhs=a4[0:2, 512 * q:512 * (q + 1)],
                                 start=True, stop=True)
                nc.scalar.add(ctall[:, 512 * q:512 * (q + 1)], cps[0:16, :], b0c[:])
            ct2set = set_p.tile([128, 256], F32, tag="ct2")
            ctv = ctall[:].rearrange("c (g t ng pb) -> c g t ng pb",
                                     g=8, t=2, ng=8, pb=16)
            for ng in range(8):
                nc.sync.dma_start(ct2set[16 * ng:16 * ng + 16, :],
                                  ctv[:, :, :, ng, :])
            # btset [(ng,c), (g,node)] = neighbor-term table for the whole set
            btset = big1_p.tile([128, 2048], F32, tag="btset")
            for q in range(4):
                bps = pl_bg.tile([128, 512], F32, tag="bgps")
                nc.tensor.matmul(bps[:], lhsT=bwrep[:],
                                 rhs=a4[0:2, 512 * q:512 * (q + 1)],
                                 start=True, stop=True)
                nc.scalar.copy(btset[:, 512 * q:512 * (q + 1)], bps[:])

            # ---------------- conv1 kNN (per graph) ----------------
            ixset1 = ix_p.tile([128, 320], I16, tag="ixs1")
            for g8 in range(8):
                for t in range(2):
                    ps = pl_sc.tile([128, 256], F32, tag="scps")
                    nc.tensor.matmul(
                        ps[:],
                        lhsT=a4[:, 256 * g8 + 128 * t:256 * g8 + 128 * (t + 1)].bitcast(F32),
                        rhs=b4[:, 256 * g8:256 * (g8 + 1)], start=True, stop=True)
                    sc = sc_p.tile([128, 256], BF16, tag="sc")
                    nc.scalar.copy(sc[:], ps[:])
                    topk20(sc, ixset1, 40 * g8 + 20 * t)
            nc.vector.tensor_tensor(out=ixset1[:].bitcast(U16),
                                    in0=ixset1[:].bitcast(U16),
                                    in1=goffs[:].bitcast(U16), op=ALU.add)

            # ---------------- conv1 gather + edge MLP ----------------
            bgall = big1_p.tile([128, 5120], F32, tag="bgall")
            nc.gpsimd.ap_gather(bgall[:], btset[:], ixset1[:],
                                channels=128, num_elems=2048, d=1, num_idxs=5120)

            x1parts = set_p.tile([128, 256], F32, tag="x1p")   # [(ng,c), (g8,t,pb)]
            prev_reduce = None
            for g8 in range(8):
                for t in range(2):
                    ctb = ct2set[:, 32 * g8 + 16 * t:32 * g8 + 16 * (t + 1)]
                    ctb = ctb.unsqueeze(1).broadcast_to((128, 20, 16))
                    h1 = h_p.tile([128, 320], F32R, tag="h1")
                    h1v = h1[:].rearrange("p (k pb) -> p k pb", k=20, pb=16)
                    bgv = bgall[:, 640 * g8 + 320 * t:640 * g8 + 320 * (t + 1)]
                    bgv = bgv.rearrange("p (k pb) -> p k pb", k=20, pb=16)
                    nc.vector.tensor_tensor(out=h1v, in0=bgv, in1=ctb, op=ALU.add)
                    nc.vector.tensor_scalar_max(out=h1[:], in0=h1[:], scalar1=0.0)
                    m1 = pl_ml.tile([128, 320], F32, tag="mlps")
                    nc.tensor.matmul(m1[:], lhsT=w1bd[:], rhs=h1[:],
                                     start=True, stop=True)
                    h2 = h_p.tile([128, 320], F32R, tag="h2")
                    nc.scalar.activation(h2[:], m1[:], AF.Relu, bias=b1rep[:])
                    if prev_reduce is not None:
                        pm2, pg8, pt = prev_reduce
                        v = pm2[:].rearrange("p (k pb) -> p pb k", k=20, pb=16)
                        nc.vector.tensor_reduce(
                            out=x1parts[:, 32 * pg8 + 16 * pt:32 * pg8 + 16 * (pt + 1)],
                            in_=v, axis=AX.X, op=ALU.max)
                    m2 = pl_ml.tile([128, 320], F32, tag="mlps")
                    nc.tensor.matmul(m2[:], lhsT=w2bd[:], rhs=h2[:],
                                     start=True, stop=True)
                    prev_reduce = (m2, g8, t)
            pm2, pg8, pt = prev_reduce
            v = pm2[:].rearrange("p (k pb) -> p pb k", k=20, pb=16)
            nc.vector.tensor_reduce(
                out=x1parts[:, 32 * pg8 + 16 * pt:32 * pg8 + 16 * (pt + 1)],
                in_=v, axis=AX.X, op=ALU.max)

            # x1 += b2 ; relayout to feature-major
            nc.vector.tensor_scalar_add(out=x1parts[:], in0=x1parts[:], scalar1=b2rep[:])
            x1t8c = set_p.tile([16, 2048], F32R, tag="x1c")    # [c, (g8,t,ng,pb)]
            x1v = x1t8c[:].rearrange("c (g t ng pb) -> c g t ng pb",
                                     g=8, t=2, ng=8, pb=16)
            for ng in range(8):
                # dst [16c, (g8,t,[ng],pb)] <- src [16c, (g8,t,pb)]
                nc.sync.dma_start(x1v[:, :, :, ng:ng + 1, :],
                                  x1parts[16 * ng:16 * ng + 16, :].bitcast(F32R))

            # ---------------- s18 assembly (conv2 kNN operands) ----------------
            # rows: s18a = [ones; x1(16); zeros], s18b = [sq; x1(16); ones]
            x1sq = set_p.tile([16, 2048], F32R, tag="x1sq")
            nc.scalar.activation(x1sq[:], x1t8c[:], AF.Square)
            s18a = s18_p.tile([18, 2048], F32, tag="s18a")
            s18b = s18_p.tile([18, 2048], F32, tag="s18b")
            nc.sync.dma_start(s18a[1:17, :], x1t8c[:].bitcast(F32))
            nc.sync.dma_start(s18a[0:1, :], b4[3:4, :])   # ones
            nc.sync.dma_start(s18a[17:18, :], a4[3:4, :].bitcast(F32))   # zeros
            nc.sync.dma_start(s18b[1:17, :], x1t8c[:].bitcast(F32))
            nc.sync.dma_start(s18b[17:18, :], b4[3:4, :])   # ones
            for q in range(4):
                mq = pl_bg.tile([128, 512], F32, tag="bgps")
                nc.tensor.matmul(mq[0:1, :], lhsT=nh16[:], rhs=x1sq[:, 512 * q:512 * (q + 1)],
                                 start=True, stop=True)
                nc.scalar.copy(s18b[0:1, 512 * q:512 * (q + 1)], mq[0:1, :])

            # ---------------- conv2 neighbor table (interleaved, bf16) ---------
            # qabset [(ng,c16), (g,node,e)]: e=0 -> out ch 0:16 (qa), e=1 -> 16:32
            qabset = big1_p.tile([128, 4096], BF16, tag="qabset")
            qabv = qabset[:].rearrange("p (j e) -> p j e", j=2048, e=2)
            for q in range(4):
                pqa = pl_bg.tile([128, 512], F32, tag="bgps")
                nc.tensor.matmul(pqa[:], lhsT=qarep_w[:],
                                 rhs=x1t8c[:, 512 * q:512 * (q + 1)], start=True, stop=True)
                nc.scalar.copy(qabv[:, 512 * q:512 * (q + 1), 0], pqa[:])
                pqb = pl_bg.tile([128, 512], F32, tag="bgps")
                nc.tensor.matmul(pqb[:], lhsT=qbrep_w[:],
                                 rhs=x1t8c[:, 512 * q:512 * (q + 1)], start=True, stop=True)
                nc.scalar.copy(qabv[:, 512 * q:512 * (q + 1), 1], pqb[:])

            f48 = f48_p.tile([48, 2048], F32R, tag="f48")
            nc.sync.dma_start(f48[32:48, :], x1t8c[:])

            # ---------------- conv2 kNN (per graph) + linear term --------------
            ixset2 = ix_p.tile([128, 320], I16, tag="ixs2")
            for g8 in range(8):
                for t in range(2):
                    ps = pl_sc.tile([128, 256], F32, tag="scps")
                    nc.tensor.matmul(
                        ps[:], lhsT=s18a[:, 256 * g8 + 128 * t:256 * g8 + 128 * (t + 1)],
                        rhs=s18b[:, 256 * g8:256 * (g8 + 1)], start=True, stop=True)
                    sc = sc_p.tile([128, 256], BF16, tag="sc")
                    nc.scalar.copy(sc[:], ps[:])
                    topk20(sc, ixset2, 40 * g8 + 20 * t)
                # linear term of conv2 (+bias) -> F48 rows 0..31 directly
                l2 = pl_bg.tile([128, 512], F32, tag="bgps")
                nc.tensor.matmul(l2[0:32, 0:256], lhsT=wd2aug[:],
                                 rhs=s18b[:, 256 * g8:256 * (g8 + 1)], start=True, stop=True)
                nc.scalar.copy(f48[0:32, 256 * g8:256 * (g8 + 1)], l2[0:32, 0:256])
            nc.vector.tensor_tensor(out=ixset2[:].bitcast(U16),
                                    in0=ixset2[:].bitcast(U16),
                                    in1=goffs[:].bitcast(U16), op=ALU.add)

            # ---------------- conv2 gather + max-aggregation -------------------
            gall = big1_p.tile([128, 10240], BF16, tag="gall")
            nc.gpsimd.ap_gather(gall[:].rearrange("p (i e) -> p i e", i=5120, e=2),
                                qabset[:].rearrange("p (j e) -> p j e", j=2048, e=2),
                                ixset2[:], channels=128, num_elems=2048, d=2, num_idxs=5120)
            x2pa = set_p.tile([128, 256], F32, tag="x2pa")   # [(ng,c16), (g,t,pb)] e=0
            x2pb = set_p.tile([128, 256], F32, tag="x2pb")   # e=1
            for g8 in range(8):
                for t in range(2):
                    gv = gall[:, 1280 * g8 + 640 * t:1280 * g8 + 640 * (t + 1)]
                    gv = gv.rearrange("p (k pb e) -> p pb e k", k=20, pb=16, e=2)
                    nc.vector.tensor_reduce(
                        out=x2pa[:, 32 * g8 + 16 * t:32 * g8 + 16 * (t + 1)],
                        in_=gv[:, :, 0, :], axis=AX.X, op=ALU.max)
                    nc.vector.tensor_reduce(
                        out=x2pb[:, 32 * g8 + 16 * t:32 * g8 + 16 * (t + 1)],
                        in_=gv[:, :, 1, :], axis=AX.X, op=ALU.max)

            # relayout aggregated max-terms, then one add into F48 rows 0..31
            x2t = big1_p.tile([32, 2048], F32, tag="x2t")
            x2tv = x2t[:].rearrange("c (g t ng pb) -> c g t ng pb",
                                    g=8, t=2, ng=8, pb=16)
            for ng in range(8):
                nc.sync.dma_start(x2tv[0:16, :, :, ng:ng + 1, :],
                                  x2pa[16 * ng:16 * ng + 16, :])
                nc.sync.dma_start(x2tv[16:32, :, :, ng:ng + 1, :],
                                  x2pb[16 * ng:16 * ng + 16, :])
            nc.vector.tensor_tensor(out=f48[0:32, :], in0=f48[0:32, :],
                                    in1=x2t[:], op=ALU.add)

            # ---------------- lin1 + global max pool ----------------
            for g8 in range(8):
                pl = pl_sc.tile([128, 256], F32, tag="scps")
                nc.tensor.matmul(pl[:], lhsT=w1l[:], rhs=f48[:, 256 * g8:256 * (g8 + 1)],
                                 start=True, stop=True)
                nc.vector.tensor_reduce(out=pooledT[:, 8 * s + g8:8 * s + g8 + 1],
                                        in_=pl[:], axis=AX.X, op=ALU.max)

        # ---------------- head MLP ----------------
        nc.vector.tensor_scalar_add(out=pooledT[:], in0=pooledT[:], scalar1=b1l[:])
        hd1 = pl_bg.tile([128, 512], F32, tag="bgps")
        nc.tensor.matmul(hd1[0:64, 0:G], lhsT=mw0[:], rhs=pooledT[:], start=True, stop=True)
        h1s = wp.tile([64, G], F32)
        nc.scalar.activation(h1s[:], hd1[0:64, 0:G], AF.Relu, bias=mb0[:])
        hd2 = pl_bg.tile([128, 512], F32, tag="bgps")
        nc.tensor.matmul(hd2[0:64, 0:G], lhsT=mw1[:], rhs=h1s[:], start=True, stop=True)
        h2s = wp.tile([64, G], F32)
        nc.scalar.activation(h2s[:], hd2[0:64, 0:G], AF.Relu, bias=mb1[:])
        hd3 = pl_bg.tile([128, 512], F32, tag="bgps")
        nc.tensor.matmul(hd3[0:1, 0:G], lhsT=mw2[:], rhs=h2s[:], start=True, stop=True)
        outs = wp.tile([1, G], F32)
        nc.vector.tensor_scalar_add(out=outs[:], in0=hd3[0:1, 0:G], scalar1=mb2[:])
        nc.sync.dma_start(out_d.ap(), outs[:])

    nc.compile()
    return nc


# ---------------------------------------------------------------------------
# Host-side input prep
# ---------------------------------------------------------------------------

def _blkdiag8(w):
    w = np.asarray(w, np.float32)
    out = np.zeros((128, 128), np.float32)
    for i in range(8):
        out[16 * i:16 * i + 16, 16 * i:16 * i + 16] = w
    return out


def make_in_maps(inputs):
    x = np.asarray(inputs["x"], np.float32)
    c1_w0 = np.asarray(inputs["c1_w0"], np.float32)
    consts = {}
    consts["cw2"] = (c1_w0[:2] - c1_w0[2:4]).astype(np.float32)              # [2,16]
    consts["b0c"] = np.asarray(inputs["c1_b0"], np.float32)[:, None]         # [16,1]
    consts["bwrep"] = np.tile(c1_w0[2:4], (1, 8)).astype(np.float32)         # [2,128]
    consts["w1bd"] = _blkdiag8(inputs["c1_w1"])
    consts["b1rep"] = np.tile(np.asarray(inputs["c1_b1"], np.float32), 8)[:, None]
    consts["w2bd"] = _blkdiag8(inputs["c1_w2"])
    consts["b2rep"] = np.tile(np.asarray(inputs["c1_b2"], np.float32), 8)[:, None]
    bdgf = np.zeros((128, 64), np.float32)
    for g in range(64):
        bdgf[2 * g, g] = -0.5
        bdgf[2 * g + 1, g] = -0.5
    consts["bdgf"] = bdgf
    consts["neghalf16"] = np.full((16, 1), -0.5, np.float32)
    consts["goffs"] = np.tile(np.repeat(np.arange(8, dtype=np.int16) * 256, 20 * 2),
                              (128, 1))                                      # [128,320]
    c2_w0 = np.asarray(inputs["c2_w0"], np.float32)
    consts["wb2repA"] = np.tile(c2_w0[16:32, 0:16], (1, 8)).astype(np.float32)
    consts["wb2repB"] = np.tile(c2_w0[16:32, 16:32], (1, 8)).astype(np.float32)
    # rows match s18b = [sq; x1(16); ones]
    wd2aug = np.zeros((18, 32), np.float32)
    wd2aug[1:17] = c2_w0[0:16] - c2_w0[16:32]
    wd2aug[17] = np.asarray(inputs["c2_b0"], np.float32)
    consts["wd2aug"] = wd2aug
    w1l = np.asarray(inputs["lin1_w"], np.float32)
    consts["w1l"] = np.concatenate([w1l[16:48], w1l[0:16]], axis=0)  # f48 row order
    consts["b1l"] = np.asarray(inputs["lin1_b"], np.float32)[:, None]
    consts["mw0"] = np.asarray(inputs["m_w0"], np.float32)
    consts["mb0"] = np.asarray(inputs["m_b0"], np.float32)[:, None]
    consts["mw1"] = np.asarray(inputs["m_w1"], np.float32)
    consts["mb1"] = np.asarray(inputs["m_b1"], np.float32)[:, None]
    consts["mw2"] = np.asarray(inputs["m_w2"], np.float32)
    consts["mb2"] = np.asarray(inputs["m_b2"], np.float32)[:, None]

    in_maps = []
    npc = N * GPC  # nodes per core
    for c in range(NCORES):
        xc = x[c * npc:(c + 1) * npc]                       # [16384, 2]
        xg = xc.reshape(GPC, N, 2)
        m = dict(consts)
        m["xtf"] = xg.transpose(0, 2, 1).reshape(128, 256).copy()   # [2g+f, j]
        rows = xc.T.reshape(2, -1)                          # [f, 256g+j]
        a4 = np.zeros((4, 16384), np.float32)
        a4[0:2] = rows
        a4[2] = 1.0
        m["a4all"] = a4
        b4 = np.zeros((4, 16384), np.float32)
        b4[0:2] = rows
        b4[3] = 1.0
        m["b4all"] = b4
        in_maps.append(m)
    return in_maps


_CACHED = {}


def _get_program(sets=8):
    if sets not in _CACHED:
        _CACHED[sets] = build_program(sets)
    return _CACHED[sets]


def run(inputs, trace=False, **kw):
    nc = _get_program(8)
    in_maps = make_in_maps(inputs)
    res = run_bass_kernel_spmd(nc, in_maps, list(range(NCORES)), trace=trace, **kw)
    out = np.concatenate([res.results[c]["out"].reshape(GPC) for c in range(NCORES)])
    return out.reshape(B, 1).astype(np.float32), res


def kernel(**inputs) -> np.ndarray:
    out, _ = run(inputs, trace=False)
    return out


# revision 57
# speedup vs baseline: 2.0363x; 1.1540x over previous
"""DGCNN-style GNN (2x dynamic-kNN EdgeConv + global pool + MLP head) on 8 Trainium2
NeuronCores, data-parallel over the 512 graphs (64 graphs per core).

Self-contained: hardcodes all shapes; host side only does layout prep (transpose /
tiling / block-diag packing of weights) and sharding.

v4: ap_gather moves ~0.65 words/ns/core, so gather volume is the pacing resource.
conv2 gathers x1_j (16 ch) instead of the 32-ch q-table (q computed post-gather by
two block-diag f32r matmuls), halving conv2 gather volume. Gathers are issued as
2-graph sub-gathers (4 per conv per set) and the per-set phases are software-
pipelined (conv2-finish of set s-1 emitted between set s's kNN and MLP phases) so
gather execution overlaps Vector/Tensor work. Non-score matmuls run in float32r.
"""

import sys

for _p in ("/opt/trn_rl_repo",):
    if _p not in sys.path:
        sys.path.append(_p)

from contextlib import ExitStack

import numpy as np

import concourse.bass as bass
import concourse.tile as tile
from concourse import bacc, mybir
from concourse.bass_utils import run_bass_kernel_spmd

F32 = mybir.dt.float32
F32R = mybir.dt.float32r
BF16 = mybir.dt.bfloat16
U16 = mybir.dt.uint16
I16 = mybir.dt.int16
AF = mybir.ActivationFunctionType
ALU = mybir.AluOpType
AX = mybir.AxisListType

B, N, K = 512, 256, 20
NCORES = 8
GPC = B // NCORES  # graphs per core = 64
NEG = -1.0e30


def build_program(sets: int = 8):
    G = 8 * sets
    nc = bacc.Bacc("TRN2", target_bir_lowering=False, debug=False)

    def din(name, shape, dtype=F32):
        return nc.declare_dram_parameter(name, list(shape), dtype, isOutput=False)

    # -------------------- DRAM parameters --------------------
    xtf_d = din("xtf", [128, 256])
    a4_d = din("a4all", [4, 16384])         # rows: x0, x1, ones, zeros
    b4_d = din("b4all", [4, 16384])         # rows: x0, x1, zeros(->-sq/2), ones
    cw2_d = din("cw2", [2, 16])             # c1_w0[:2] - c1_w0[2:4]
    b0c_d = din("b0c", [16, 1])             # c1_b0
    bwrep_d = din("bwrep", [2, 128])        # tile8(c1_w0[2:4])
    w1bd_d = din("w1bd", [128, 128])        # blkdiag8(c1_w1)
    b1rep_d = din("b1rep", [128, 1])
    w2bd_d = din("w2bd", [128, 128])        # blkdiag8(c1_w2)
    b2rep_d = din("b2rep", [128, 1])
    bdgf_d = din("bdgf", [128, 64])
    nh16_d = din("neghalf16", [16, 1])
    goffs_d = din("goffs", [128, 320], I16)  # [:, 40g+j] = 256*g
    qbd_a_d = din("qbdA", [128, 128])       # blkdiag8(c2_w0[16:32, :16])
    qbd_b_d = din("qbdB", [128, 128])       # blkdiag8(c2_w0[16:32, 16:])
    wd2aug_d = din("wd2aug", [18, 32])      # rows match s18b = [sq; x1; ones]
    w1l_d = din("w1l", [48, 128])           # lin1_w, rows = [x2(32); x1(16)]
    b1l_d = din("b1l", [128, 1])
    mw0_d = din("mw0", [128, 64])
    mb0_d = din("mb0", [64, 1])
    mw1_d = din("mw1", [64, 64])
    mb1_d = din("mb1", [64, 1])
    mw2_d = din("mw2", [64, 1])
    mb2_d = din("mb2", [1, 1])
    out_d = nc.declare_dram_parameter("out", [1, G], F32, isOutput=True)

    with tile.TileContext(nc) as tc, ExitStack() as ctx:
        P = lambda **kw: ctx.enter_context(tc.tile_pool(**kw))
        wp = P(name="weights", bufs=1)

        def load(dram, shape, dtype=F32):
            t = wp.tile(list(shape), dtype, tag=dram.name)
            src = dram.ap()
            if dtype == F32R:
                src = src.bitcast(F32R)
            nc.sync.dma_start(t[:], src)
            return t

        xtf = load(xtf_d, [128, 256])
        cw2 = load(cw2_d, [2, 16], F32R)
        b0c = load(b0c_d, [16, 1])
        bwrep = load(bwrep_d, [2, 128], F32R)
        w1bd = load(w1bd_d, [128, 128], F32R)
        b1rep = load(b1rep_d, [128, 1])
        w2bd = load(w2bd_d, [128, 128], F32R)
        b2rep = load(b2rep_d, [128, 1])
        bdgf = load(bdgf_d, [128, 64])
        nh16 = load(nh16_d, [16, 1], F32R)
        goffs = load(goffs_d, [128, 320], I16)
        qbd_a = load(qbd_a_d, [128, 128])
        qbd_b = load(qbd_b_d, [128, 128])
        wd2aug = load(wd2aug_d, [18, 32])
        w1l = load(w1l_d, [48, 128], F32R)
        b1l = load(b1l_d, [128, 1])
        mw0 = load(mw0_d, [128, 64])
        mb0 = load(mb0_d, [64, 1])
        mw1 = load(mw1_d, [64, 64])
        mb1 = load(mb1_d, [64, 1])
        mw2 = load(mw2_d, [64, 1])
        mb2 = load(mb2_d, [1, 1])

        pooledT = wp.tile([128, G], F32)

        # PSUM pools: 2 + 2 + 4 banks = 8
        pl_sc = P(name="scps", bufs=2, space="PSUM")      # [128,256] score psums
        pl_ml = P(name="mlps", bufs=2, space="PSUM")      # [128,320] mlp/q psums
        pl_bg = P(name="bgps", bufs=4, space="PSUM")      # [128,512] everything else

        sc_p = P(name="scores", bufs=3)
        v_p = P(name="vals8", bufs=3)
        ix_p = P(name="idx", bufs=6)
        h_p = P(name="hid", bufs=3)
        set_p = P(name="sets", bufs=2)
        s18_p = P(name="s18", bufs=1)
        f48_p = P(name="f48", bufs=2)
        big1_p = P(name="big1", bufs=1)
        gp_p = P(name="gath", bufs=4)       # 2-graph gather outputs [128,1280]

        # ---------------- global prep: sqG = -0.5*(x0^2+x1^2) [64,256] ----------
        xsq = sc_p.tile([128, 256], F32, tag="xsq")
        nc.vector.tensor_tensor(out=xsq[:], in0=xtf[:], in1=xtf[:], op=ALU.mult)
        sq_ps = pl_bg.tile([128, 512], F32, tag="bgps")
        nc.tensor.matmul(sq_ps[0:64, 0:256], lhsT=bdgf[:], rhs=xsq[:], start=True, stop=True)
        sqG = wp.tile([64, 256], F32)
        nc.scalar.copy(sqG[:], sq_ps[0:64, 0:256])

        def topk20(scores_sb, ixp, col0):
            """scores_sb [128,256] bf16 (destroyed); writes indices of ranks 1..20
            into ixp[:, col0:col0+20] as i16."""
            ixt = ix_p.tile([128, 24], U16, tag="ix")
            for r in range(3):
                v = v_p.tile([128, 8], BF16, tag="v8")
                nc.vector.max(v[:], scores_sb[:])
                nc.vector.max_index(ixt[:, 8 * r:8 * r + 8], v[:], scores_sb[:])
                if r < 2:
                    nc.vector.match_replace(scores_sb[:], v[:], scores_sb[:], NEG)
            nc.vector.tensor_copy(out=ixp[:, col0:col0 + 20], in_=ixt[:, 1:21])

        # per-set live state carried between pipelined phases
        st = [dict() for _ in range(sets)]

        # ------------------------------------------------------------------
        # Phase A(s): conv1 tables + kNN + 4 sub-gather issues
        # ------------------------------------------------------------------
        def phase_A(s):
            d = st[s]
            a4 = set_p.tile([4, 2048], F32R, tag="a4")
            nc.sync.dma_start(a4[:], a4_d.ap()[:, 2048 * s:2048 * (s + 1)].bitcast(F32R))
            b4 = set_p.tile([4, 2048], F32, tag="b4")
            nc.sync.dma_start(b4[:], b4_d.ap()[:, 2048 * s:2048 * (s + 1)])
            nc.sync.dma_start(b4[2:3, :], sqG[8 * s:8 * s + 8, :])
            d["a4"], d["b4"] = a4, b4

            # center-term table ct2set [(ng,c), (g,t,pb)]
            ctall = big1_p.tile([16, 2048], F32, tag="ctall")
            for q in range(4):
                cps = pl_bg.tile([128, 512], F32, tag="bgps")
                nc.tensor.matmul(cps[0:16, :], lhsT=cw2[:],
                                 rhs=a4[0:2, 512 * q:512 * (q + 1)], start=True, stop=True)
                nc.scalar.add(ctall[:, 512 * q:512 * (q + 1)], cps[0:16, :], b0c[:])
            ct2set = set_p.tile([128, 256], F32, tag="ct2")
            ctv = ctall[:].rearrange("c (g t ng pb) -> c g t ng pb",
                                     g=8, t=2, ng=8, pb=16)
            for ng in range(8):
                nc.sync.dma_start(ct2set[16 * ng:16 * ng + 16, :], ctv[:, :, :, ng, :])
            d["ct2set"] = ct2set

            # neighbor-term table btset [(ng,c), (g,node)]
            btset = big1_p.tile([128, 2048], F32, tag="btset")
            for q in range(4):
                bps = pl_bg.tile([128, 512], F32, tag="bgps")
                nc.tensor.matmul(bps[:], lhsT=bwrep[:],
                                 rhs=a4[0:2, 512 * q:512 * (q + 1)], start=True, stop=True)
                nc.scalar.copy(btset[:, 512 * q:512 * (q + 1)], bps[:])

            d["bg"] = []
            for p in range(4):          # 2-graph pairs
                ixp = ix_p.tile([128, 80], I16, tag="ixp1")
                for gg in range(2):
                    g8 = 2 * p + gg
                    for t in range(2):
                        ps = pl_sc.tile([128, 256], F32, tag="scps")
                        nc.tensor.matmul(
                            ps[:],
                            lhsT=a4[:, 256 * g8 + 128 * t:256 * g8 + 128 * (t + 1)].bitcast(F32),
                            rhs=b4[:, 256 * g8:256 * (g8 + 1)], start=True, stop=True)
                        sc = sc_p.tile([128, 256], BF16, tag="sc")
                        nc.scalar.copy(sc[:], ps[:])
                        topk20(sc, ixp, 40 * gg + 20 * t)
                nc.vector.tensor_tensor(out=ixp[:].bitcast(U16), in0=ixp[:].bitcast(U16),
                                        in1=goffs[:, 80 * p:80 * (p + 1)].bitcast(U16),
                                        op=ALU.add)
                bgp = gp_p.tile([128, 1280], F32, tag="bgp")
                nc.gpsimd.ap_gather(bgp[:], btset[:], ixp[:],
                                    channels=128, num_elems=2048, d=1, num_idxs=1280)
                d["bg"].append(bgp)

        # ------------------------------------------------------------------
        # Phase B1(s): conv1 edge MLP (consumes sub-gathers)
        # ------------------------------------------------------------------
        def phase_B1(s):
            d = st[s]
            ct2set = d["ct2set"]
            x1parts = set_p.tile([128, 256], F32, tag="x1p")
            prev = None
            for p in range(4):
                bgp = d["bg"][p]
                for gg in range(2):
                    g8 = 2 * p + gg
                    for t in range(2):
                        ctb = ct2set[:, 32 * g8 + 16 * t:32 * g8 + 16 * (t + 1)]
                        ctb = ctb.unsqueeze(1).broadcast_to((128, 20, 16))
                        h1 = h_p.tile([128, 320], F32R, tag="h1")
                        h1v = h1[:].rearrange("p (k pb) -> p k pb", k=20, pb=16)
                        bgv = bgp[:, 640 * gg + 320 * t:640 * gg + 320 * (t + 1)]
                        bgv = bgv.rearrange("p (k pb) -> p k pb", k=20, pb=16)
                        nc.vector.tensor_tensor(out=h1v, in0=bgv, in1=ctb, op=ALU.add)
                        nc.vector.tensor_scalar_max(out=h1[:], in0=h1[:], scalar1=0.0)
                        m1 = pl_ml.tile([128, 320], F32, tag="mlps")
                        nc.tensor.matmul(m1[:], lhsT=w1bd[:], rhs=h1[:], start=True, stop=True)
                        h2 = h_p.tile([128, 320], F32R, tag="h2")
                        nc.scalar.activation(h2[:], m1[:], AF.Relu, bias=b1rep[:])
                        if prev is not None:
                            pm, pg, pt = prev
                            v = pm[:].rearrange("p (k pb) -> p pb k", k=20, pb=16)
                            nc.vector.tensor_reduce(
                                out=x1parts[:, 32 * pg + 16 * pt:32 * pg + 16 * (pt + 1)],
                                in_=v, axis=AX.X, op=ALU.max)
                        m2 = pl_ml.tile([128, 320], F32, tag="mlps")
                        nc.tensor.matmul(m2[:], lhsT=w2bd[:], rhs=h2[:], start=True, stop=True)
                        prev = (m2, g8, t)
            pm, pg, pt = prev
            v = pm[:].rearrange("p (k pb) -> p pb k", k=20, pb=16)
            nc.vector.tensor_reduce(
                out=x1parts[:, 32 * pg + 16 * pt:32 * pg + 16 * (pt + 1)],
                in_=v, axis=AX.X, op=ALU.max)
            nc.vector.tensor_scalar_add(out=x1parts[:], in0=x1parts[:], scalar1=b2rep[:])
            d["x1parts"] = x1parts

        # ------------------------------------------------------------------
        # Phase B2(s): x1 relayout, s18 assembly, x1rep, conv2 kNN + gathers
        # ------------------------------------------------------------------
        def phase_B2(s):
            d = st[s]
            a4, b4, x1parts = d["a4"], d["b4"], d["x1parts"]
            x1t8c = set_p.tile([16, 2048], F32R, tag="x1c")
            x1v = x1t8c[:].rearrange("c (g t ng pb) -> c g t ng pb",
                                     g=8, t=2, ng=8, pb=16)
            for ng in range(8):
                nc.sync.dma_start(x1v[:, :, :, ng:ng + 1, :],
                                  x1parts[16 * ng:16 * ng + 16, :].bitcast(F32R))
            x1sq = set_p.tile([16, 2048], F32R, tag="x1sq")
            nc.scalar.activation(x1sq[:], x1t8c[:], AF.Square)
            s18a = s18_p.tile([18, 2048], F32, tag="s18a")
            s18b = s18_p.tile([18, 2048], F32, tag="s18b")
            nc.sync.dma_start(s18a[1:17, :], x1t8c[:].bitcast(F32))
            nc.sync.dma_start(s18a[0:1, :], b4[3:4, :])   # ones
            nc.sync.dma_start(s18a[17:18, :], a4[3:4, :].bitcast(F32))   # zeros
            nc.sync.dma_start(s18b[1:17, :], x1t8c[:].bitcast(F32))
            nc.sync.dma_start(s18b[17:18, :], b4[3:4, :])   # ones
            for q in range(4):
                mq = pl_bg.tile([128, 512], F32, tag="bgps")
                nc.tensor.matmul(mq[0:1, :], lhsT=nh16[:],
                                 rhs=x1sq[:, 512 * q:512 * (q + 1)], start=True, stop=True)
                nc.scalar.copy(s18b[0:1, 512 * q:512 * (q + 1)], mq[0:1, :])
            d["s18a"], d["s18b"] = s18a, s18b

            # x1rep [(ng,c), (g,node)] = x1 replicated into all 8 row-blocks
            x1rep = big1_p.tile([128, 2048], F32, tag="x1rep")
            for ng in range(8):
                nc.sync.dma_start(x1rep[16 * ng:16 * ng + 16, :], x1t8c[:].bitcast(F32))

            f48 = f48_p.tile([48, 2048], F32R, tag="f48")
            nc.sync.dma_start(f48[32:48, :], x1t8c[:])
            d["f48"] = f48

            d["xj"] = []
            for p in range(4):
                ixp = ix_p.tile([128, 80], I16, tag="ixp2")
                for gg in range(2):
                    g8 = 2 * p + gg
                    for t in range(2):
                        ps = pl_sc.tile([128, 256], F32, tag="scps")
                        nc.tensor.matmul(
                            ps[:], lhsT=s18a[:, 256 * g8 + 128 * t:256 * g8 + 128 * (t + 1)],
                            rhs=s18b[:, 256 * g8:256 * (g8 + 1)], start=True, stop=True)
                        sc = sc_p.tile([128, 256], BF16, tag="sc")
                        nc.scalar.copy(sc[:], ps[:])
                        topk20(sc, ixp, 40 * gg + 20 * t)
                nc.vector.tensor_tensor(out=ixp[:].bitcast(U16), in0=ixp[:].bitcast(U16),
                                        in1=goffs[:, 80 * p:80 * (p + 1)].bitcast(U16),
                                        op=ALU.add)
                xjp = gp_p.tile([128, 1280], F32, tag="xjp")
                nc.gpsimd.ap_gather(xjp[:], x1rep[:], ixp[:],
                                    channels=128, num_elems=2048, d=1, num_idxs=1280)
                d["xj"].append(xjp)

        # ------------------------------------------------------------------
        # Phase C(s): conv2 q-matmuls + aggregation + f48 + lin1 + pool
        # ------------------------------------------------------------------
        def phase_C(s):
            d = st[s]
            s18b, f48 = d["s18b"], d["f48"]
            x2pa = set_p.tile([128, 256], F32, tag="x2pa")
            x2pb = set_p.tile([128, 256], F32, tag="x2pb")
            for p in range(4):
                xjp = d["xj"][p]
                for gg in range(2):
                    g8 = 2 * p + gg
                    for t in range(2):
                        xv = xjp[:, 640 * gg + 320 * t:640 * gg + 320 * (t + 1)]
                        qa = pl_ml.tile([128, 320], F32, tag="mlps")
                        nc.tensor.matmul(qa[:], lhsT=qbd_a[:], rhs=xv, start=True, stop=True)
                        qv = qa[:].rearrange("p (k pb) -> p pb k", k=20, pb=16)
                        nc.vector.tensor_reduce(
                            out=x2pa[:, 32 * g8 + 16 * t:32 * g8 + 16 * (t + 1)],
                            in_=qv, axis=AX.X, op=ALU.max)
                        qb = pl_ml.tile([128, 320], F32, tag="mlps")
                        nc.tensor.matmul(qb[:], lhsT=qbd_b[:], rhs=xv, start=True, stop=True)
                        qv = qb[:].rearrange("p (k pb) -> p pb k", k=20, pb=16)
                        nc.vector.tensor_reduce(
                            out=x2pb[:, 32 * g8 + 16 * t:32 * g8 + 16 * (t + 1)],
                            in_=qv, axis=AX.X, op=ALU.max)
            # linear term of conv2 (+bias) -> F48 rows 0..31 directly
            for g8 in range(8):
                l2 = pl_bg.tile([128, 512], F32, tag="bgps")
                nc.tensor.matmul(l2[0:32, 0:256], lhsT=wd2aug[:],
                                 rhs=s18b[:, 256 * g8:256 * (g8 + 1)], start=True, stop=True)
                nc.scalar.copy(f48[0:32, 256 * g8:256 * (g8 + 1)], l2[0:32, 0:256])
            x2t = big1_p.tile([32, 2048], F32, tag="x2t")
            x2tv = x2t[:].rearrange("c (g t ng pb) -> c g t ng pb",
                                    g=8, t=2, ng=8, pb=16)
            for ng in range(8):
                nc.sync.dma_start(x2tv[0:16, :, :, ng:ng + 1, :],
                                  x2pa[16 * ng:16 * ng + 16, :])
                nc.sync.dma_start(x2tv[16:32, :, :, ng:ng + 1, :],
                                  x2pb[16 * ng:16 * ng + 16, :])
            nc.vector.tensor_tensor(out=f48[0:32, :], in0=f48[0:32, :],
                                    in1=x2t[:], op=ALU.add)
            for g8 in range(8):
                pl = pl_sc.tile([128, 256], F32, tag="scps")
                nc.tensor.matmul(pl[:], lhsT=w1l[:], rhs=f48[:, 256 * g8:256 * (g8 + 1)],
                                 start=True, stop=True)
                nc.vector.tensor_reduce(out=pooledT[:, 8 * s + g8:8 * s + g8 + 1],
                                        in_=pl[:], axis=AX.X, op=ALU.max)
            st[s] = {}

        # ---------------- pipelined emission ----------------
        for s in range(sets):
            phase_A(s)
            if s > 0:
                phase_C(s - 1)
            phase_B1(s)
            phase_B2(s)
        phase_C(sets - 1)

        # ---------------- head MLP ----------------
        nc.vector.tensor_scalar_add(out=pooledT[:], in0=pooledT[:], scalar1=b1l[:])
        hd1 = pl_bg.tile([128, 512], F32, tag="bgps")
        nc.tensor.matmul(hd1[0:64, 0:G], lhsT=mw0[:], rhs=pooledT[:], start=True, stop=True)
        h1s = wp.tile([64, G], F32)
        nc.scalar.activation(h1s[:], hd1[0:64, 0:G], AF.Relu, bias=mb0[:])
        hd2 = pl_bg.tile([128, 512], F32, tag="bgps")
        nc.tensor.matmul(hd2[0:64, 0:G], lhsT=mw1[:], rhs=h1s[:], start=True, stop=True)
        h2s = wp.tile([64, G], F32)
        nc.scalar.activation(h2s[:], hd2[0:64, 0:G], AF.Relu, bias=mb1[:])
        hd3 = pl_bg.tile([128, 512], F32, tag="bgps")
        nc.tensor.matmul(hd3[0:1, 0:G], lhsT=mw2[:], rhs=h2s[:], start=True, stop=True)
        outs = wp.tile([1, G], F32)
        nc.vector.tensor_scalar_add(out=outs[:], in0=hd3[0:1, 0:G], scalar1=mb2[:])
        nc.sync.dma_start(out_d.ap(), outs[:])

    nc.compile()
    return nc


# ---------------------------------------------------------------------------
# Host-side input prep
# ---------------------------------------------------------------------------

def _blkdiag8(w):
    w = np.asarray(w, np.float32)
    n, m = w.shape
    out = np.zeros((8 * n, 8 * m), np.float32)
    for i in range(8):
        out[n * i:n * i + n, m * i:m * i + m] = w
    return out


def make_in_maps(inputs):
    x = np.asarray(inputs["x"], np.float32)
    c1_w0 = np.asarray(inputs["c1_w0"], np.float32)
    consts = {}
    consts["cw2"] = (c1_w0[:2] - c1_w0[2:4]).astype(np.float32)
    consts["b0c"] = np.asarray(inputs["c1_b0"], np.float32)[:, None]
    consts["bwrep"] = np.tile(c1_w0[2:4], (1, 8)).astype(np.float32)
    consts["w1bd"] = _blkdiag8(inputs["c1_w1"])
    consts["b1rep"] = np.tile(np.asarray(inputs["c1_b1"], np.float32), 8)[:, None]
    consts["w2bd"] = _blkdiag8(inputs["c1_w2"])
    consts["b2rep"] = np.tile(np.asarray(inputs["c1_b2"], np.float32), 8)[:, None]
    bdgf = np.zeros((128, 64), np.float32)
    for g in range(64):
        bdgf[2 * g, g] = -0.5
        bdgf[2 * g + 1, g] = -0.5
    consts["bdgf"] = bdgf
    consts["neghalf16"] = np.full((16, 1), -0.5, np.float32)
    consts["goffs"] = np.tile(np.repeat(np.arange(8, dtype=np.int16) * 256, 40),
                              (128, 1))
    c2_w0 = np.asarray(inputs["c2_w0"], np.float32)
    consts["qbdA"] = _blkdiag8(c2_w0[16:32, 0:16])
    consts["qbdB"] = _blkdiag8(c2_w0[16:32, 16:32])
    wd2aug = np.zeros((18, 32), np.float32)
    wd2aug[1:17] = c2_w0[0:16] - c2_w0[16:32]
    wd2aug[17] = np.asarray(inputs["c2_b0"], np.float32)
    consts["wd2aug"] = wd2aug
    w1l = np.asarray(inputs["lin1_w"], np.float32)
    consts["w1l"] = np.concatenate([w1l[16:48], w1l[0:16]], axis=0)
    consts["b1l"] = np.asarray(inputs["lin1_b"], np.float32)[:, None]
    consts["mw0"] = np.asarray(inputs["m_w0"], np.float32)
    consts["mb0"] = np.asarray(inputs["m_b0"], np.float32)[:, None]
    consts["mw1"] = np.asarray(inputs["m_w1"], np.float32)
    consts["mb1"] = np.asarray(inputs["m_b1"], np.float32)[:, None]
    consts["mw2"] = np.asarray(inputs["m_w2"], np.float32)
    consts["mb2"] = np.asarray(inputs["m_b2"], np.float32)[:, None]

    in_maps = []
    npc = N * GPC
    for c in range(NCORES):
        xc = x[c * npc:(c + 1) * npc]
        xg = xc.reshape(GPC, N, 2)
        m = dict(consts)
        m["xtf"] = xg.transpose(0, 2, 1).reshape(128, 256).copy()
        rows = xc.T.reshape(2, -1)
        a4 = np.zeros((4, 16384), np.float32)
        a4[0:2] = rows
        a4[2] = 1.0
        m["a4all"] = a4
        b4 = np.zeros((4, 16384), np.float32)
        b4[0:2] = rows
        b4[3] = 1.0
        m["b4all"] = b4
        in_maps.append(m)
    return in_maps


_CACHED = {}


def _get_program(sets=8):
    if sets not in _CACHED:
        _CACHED[sets] = build_program(sets)
    return _CACHED[sets]


def run(inputs, trace=False, **kw):
    nc = _get_program(8)
    in_maps = make_in_maps(inputs)
    res = run_bass_kernel_spmd(nc, in_maps, list(range(NCORES)), trace=trace, **kw)
    out = np.concatenate([res.results[c]["out"].reshape(GPC) for c in range(NCORES)])
    return out.reshape(B, 1).astype(np.float32), res


def kernel(**inputs) -> np.ndarray:
    out, _ = run(inputs, trace=False)
    return out
